# revision 7
# baseline (speedup 1.0000x reference)
"""Trainium2 Bass kernel for nn_Attention_57423712748130.

Computation (per batch b):
  X4 = x[b] viewed (C=256, N=4096)   [raw reshape]
  Q4 = silu(BN(q_w @ X4))            (256, 4096)
  KV4 = silu(BN(kv_w @ X4y))         (128, 4096)
  q[n,h,d]  = Q4[n1, n0*256+h*64+d]      n = n1*16+n0
  k[m,h,d]  = KV4[m1, m0*512 + h*64+d]   m = m1*8+m0
  v[m,h,d]  = KV4[m1, m0*512+256+h*64+d]
  att = softmax(q k^T / 8); o = att v
  out rows [h*1024,(h+1)*1024) = O_h @ proj_w.T + proj_b
    where O_h[n2, n3*64+d] = o[4*n2+n3, d]

Sharding: 8 cores = (batch b in 0..3) x (head-pair hp in 0..1).
Each core computes heads {2hp, 2hp+1} of batch b and produces rows
[hp*2048, (hp+1)*2048) of out[b].

Strategy on-core: compute conv outputs directly in transposed layout
(Q4T/KV4T chunks) via matmuls with x/y tiles as the stationary operand, so
q^T/k^T slices ([d on partitions]) come out of PSUM with zero transposes.
Scores are computed transposed (scoresT[m, n] = k^T.T @ q^T), softmax-exp on
ScalarE with the 1/8 scale folded in (max-subtraction provably unneeded:
scaled scores are < 14), the sum over m comes from an extra all-ones column
appended to V in the att@v matmul, and normalization + the O_h layout
rearrangement happen on the way out. All matmuls run in float32r
(TF32-like, full PE rate at N>=256).
"""

import numpy as np

B = 4
N_TOK = 4096
C = 256
NH = 4
HD = 64
M_TOK = 1024
CKV = 128
BN_EPS = 1e-5

_CACHE = {}


def _build():
    import concourse.bacc as bacc
    import concourse.tile as tile
    from concourse import mybir

    f32 = mybir.dt.float32
    f32r = mybir.dt.float32r
    AF = mybir.ActivationFunctionType

    nc = bacc.Bacc("TRN2", target_bir_lowering=False, debug=False, num_devices=8)

    xq = nc.dram_tensor("xq", [256, 2048], f32, kind="ExternalInput")
    yk = nc.dram_tensor("yk", [256, 1024], f32, kind="ExternalInput")
    yv = nc.dram_tensor("yv", [256, 1024], f32, kind="ExternalInput")
    wq = nc.dram_tensor("wq", [256, 256], f32, kind="ExternalInput")
    bq = nc.dram_tensor("bq", [1, 256], f32, kind="ExternalInput")
    wkv = nc.dram_tensor("wkv", [256, 128], f32, kind="ExternalInput")
    bkv = nc.dram_tensor("bkv", [1, 128], f32, kind="ExternalInput")
    wp = nc.dram_tensor("wp", [256, 256], f32, kind="ExternalInput")
    bp = nc.dram_tensor("bp", [1, 256], f32, kind="ExternalInput")
    onesd = nc.dram_tensor("onesd", [1, 1040], f32, kind="ExternalInput")
    out = nc.dram_tensor("out", [2048, 256], f32, kind="ExternalOutput")
    rscr = nc.dram_tensor("rscr", [2, 4096], f32)

    import concourse.bass as bass

    with tile.TileContext(nc) as tc:
        with (
            tc.tile_pool(name="const", bufs=1) as cp,
            tc.tile_pool(name="attp", bufs=4) as attp,
            tc.tile_pool(name="outp", bufs=3) as outp,
            tc.tile_pool(name="psc", bufs=2, space="PSUM") as psc,
            tc.tile_pool(name="pss", bufs=2, space="PSUM") as pss,
            tc.tile_pool(name="pso", bufs=2, space="PSUM") as pso,
        ):
            # ---- load weights / inputs ----
            def load(t_dram, shape, tag, rows=None):
                t = cp.tile(shape, f32r, tag=tag, name=tag)
                src = t_dram.ap().bitcast(f32r)
                if rows is not None:
                    src = src[rows[0] : rows[1], :]
                nc.sync.dma_start(t[:], src)
                return t

            wq_sb = [load(wq, [128, 256], f"wq{i}", (i * 128, (i + 1) * 128)) for i in range(2)]
            wkv_sb = [load(wkv, [128, 128], f"wkv{i}", (i * 128, (i + 1) * 128)) for i in range(2)]
            wp_sb = [load(wp, [128, 256], f"wp{i}", (i * 128, (i + 1) * 128)) for i in range(2)]
            bq_sb = load(bq, [1, 256], "bq")
            bkv_sb = load(bkv, [1, 128], "bkv")
            bp_sb = load(bp, [1, 256], "bp")
            xq_sb = [load(xq, [128, 2048], f"xq{i}", (i * 128, (i + 1) * 128)) for i in range(2)]
            yk_sb = [load(yk, [128, 1024], f"yk{i}", (i * 128, (i + 1) * 128)) for i in range(2)]
            yv_sb = [load(yv, [128, 1024], f"yv{i}", (i * 128, (i + 1) * 128)) for i in range(2)]

            ones_sb = cp.tile([1, 512], f32r, tag="ones")
            nc.sync.dma_start(ones_sb[:], onesd.ap().bitcast(f32r)[:, 0:512])

            # ---- kv convs ----
            # kT[pp, m0, m1] : pp = hl*64+d rows of KV4T for this head-pair
            kT = cp.tile([128, 8, 128], f32r, tag="kT")
            for m0 in range(8):
                ps = psc.tile([128, 512], f32, tag="cnv")
                nc.tensor.matmul(
                    ps[:, :128], lhsT=yk_sb[0][:, m0 * 128 : (m0 + 1) * 128],
                    rhs=wkv_sb[0][:], start=True, stop=False)
                nc.tensor.matmul(
                    ps[:, :128], lhsT=yk_sb[1][:, m0 * 128 : (m0 + 1) * 128],
                    rhs=wkv_sb[1][:], start=False, stop=False)
                nc.tensor.matmul(
                    ps[:, :128], lhsT=ones_sb[0:1, 0:128], rhs=bkv_sb[:],
                    start=False, stop=True)
                nc.scalar.activation(kT[:, m0, :], ps[:, :128], AF.Silu)

            # vext[m1, m0, hl, 0:64] = v ; [..., 64] = 1.0 (sum column)
            vext = cp.tile([128, 8, 2, 65], f32r, tag="vext")
            nc.sync.dma_start(
                vext[:], onesd.ap().bitcast(f32r).partition_broadcast(128))
            for jv in range(2):
                ps = psc.tile([128, 512], f32, tag="cnv")
                nc.tensor.matmul(
                    ps[:], lhsT=wkv_sb[0][:], rhs=yv_sb[0][:, jv * 512 : (jv + 1) * 512],
                    start=True, stop=False)
                nc.tensor.matmul(
                    ps[:], lhsT=wkv_sb[1][:], rhs=yv_sb[1][:, jv * 512 : (jv + 1) * 512],
                    start=False, stop=False)
                nc.tensor.matmul(
                    ps[:], lhsT=bkv_sb[:], rhs=ones_sb[0:1, 0:512],
                    start=False, stop=True)
                nc.scalar.activation(
                    vext[:, jv * 4 : (jv + 1) * 4, :, 0:64],
                    ps[:].rearrange("p (a h d) -> p a h d", a=4, h=2),
                    AF.Silu)

            # ---- q conv ----  qT[pp, n0, n1]
            qT = cp.tile([128, 16, 256], f32r, tag="qT")
            for n0 in range(16):
                ps = psc.tile([128, 512], f32, tag="cnv")
                nc.tensor.matmul(
                    ps[:, :256], lhsT=xq_sb[0][:, n0 * 128 : (n0 + 1) * 128],
                    rhs=wq_sb[0][:], start=True, stop=False)
                nc.tensor.matmul(
                    ps[:, :256], lhsT=xq_sb[1][:, n0 * 128 : (n0 + 1) * 128],
                    rhs=wq_sb[1][:], start=False, stop=False)
                nc.tensor.matmul(
                    ps[:, :256], lhsT=ones_sb[0:1, 0:128], rhs=bq_sb[:],
                    start=False, stop=True)
                nc.scalar.activation(qT[:, n0, :], ps[:, :256], AF.Silu)

            # ---- attention ----
            for hl in range(2):
                r0, r1 = hl * 64, (hl + 1) * 64
                recip_all = cp.tile([1, 4096], f32, tag=f"recip{hl}")
                outun = [cp.tile([128, 1024], f32, tag=f"outun{hl}_{i}", name=f"outun{hl}_{i}") for i in range(2)]
                for n0 in range(16):
                    n3, q0 = n0 & 3, n0 >> 2
                    att = []
                    for j in range(2):
                        scp = pss.tile([128, 1024], f32, tag="scp")
                        for mm in range(4):
                            m0 = j * 4 + mm
                            nc.tensor.matmul(
                                scp[:, mm * 256 : (mm + 1) * 256],
                                lhsT=kT[r0:r1, m0, :], rhs=qT[r0:r1, n0, :],
                                start=True, stop=True)
                        a = attp.tile([128, 1024], f32r, tag="att", name=f"att{hl}_{n0}_{j}")
                        nc.scalar.activation(a[:], scp[:], AF.Exp, scale=0.125)
                        att.append(a)
                    ops = pso.tile([65, 256], f32, tag="ops")
                    for m0 in range(8):
                        nc.tensor.matmul(
                            ops[:], lhsT=vext[:, m0, hl, :],
                            rhs=att[m0 // 4][:, (m0 % 4) * 256 : (m0 % 4 + 1) * 256],
                            start=(m0 == 0), stop=(m0 == 7))
                    # store 1/sum at positions n3*1024 + n1*4 + q0 (= (n3, f)
                    # order) so the grid broadcast DMA reads contiguously
                    rdst = recip_all[:].rearrange(
                        "p (n3 n1 q) -> p n3 n1 q", n3=4, q=4)[0:1, n3, :, q0]
                    nc.vector.reciprocal(rdst, ops[64:65, :])
                    dst = outun[n3 // 2][(n3 % 2) * 64 : (n3 % 2) * 64 + 64, :]
                    dst = dst.rearrange("p (n1 q) -> p n1 q", q=4)[:, :, q0]
                    nc.vector.tensor_copy(dst.bitcast(f32r), ops[0:64, :])

                # normalization grids via DRAM bounce broadcast
                nc.sync.dma_start(rscr.ap()[hl : hl + 1, :], recip_all[:])
                for c0 in range(2):
                    grid = cp.tile([128, 1024], f32, tag=f"grid{hl}_{c0}")
                    for bnd in range(2):
                        n3g = c0 * 2 + bnd
                        src = bass.AP(
                            tensor=rscr, offset=hl * 4096 + n3g * 1024,
                            ap=[[0, 64], [1, 1024]])
                        nc.sync.dma_start(grid[bnd * 64 : (bnd + 1) * 64, :], src)
                    nc.vector.tensor_mul(
                        outun[c0][:].bitcast(f32r), outun[c0][:], grid[:])

                # proj
                for fc in range(8):
                    ps = psc.tile([128, 512], f32, tag="cnv")
                    nc.tensor.matmul(
                        ps[:, :256],
                        lhsT=outun[0][:].bitcast(f32r)[:, fc * 128 : (fc + 1) * 128],
                        rhs=wp_sb[0][:], start=True, stop=False)
                    nc.tensor.matmul(
                        ps[:, :256],
                        lhsT=outun[1][:].bitcast(f32r)[:, fc * 128 : (fc + 1) * 128],
                        rhs=wp_sb[1][:], start=False, stop=False)
                    nc.tensor.matmul(
                        ps[:, :256], lhsT=ones_sb[0:1, 0:128], rhs=bp_sb[:],
                        start=False, stop=True)
                    osb = outp.tile([128, 256], f32, tag="osb")
                    nc.vector.tensor_copy(osb[:], ps[:, :256])
                    row = hl * 1024 + fc * 128
                    nc.sync.dma_start(out.ap()[row : row + 128, :], osb[:])

    nc.compile()
    return nc


def _prep_inputs(x, y, q_w, q_gamma, q_beta, q_mean, q_var,
                 kv_w, kv_gamma, kv_beta, kv_mean, kv_var, proj_w, proj_b):
    f = np.float32
    x = np.ascontiguousarray(np.asarray(x, f))
    y = np.ascontiguousarray(np.asarray(y, f))

    gq = np.asarray(q_gamma, f) / np.sqrt(np.asarray(q_var, f) + BN_EPS)
    bq_full = np.asarray(q_beta, f) - np.asarray(q_mean, f) * gq
    wq_host = np.ascontiguousarray((np.asarray(q_w, f) * gq[:, None]).T)

    gkv = np.asarray(kv_gamma, f) / np.sqrt(np.asarray(kv_var, f) + BN_EPS)
    bkv_full = np.asarray(kv_beta, f) - np.asarray(kv_mean, f) * gkv
    wkv_host = np.ascontiguousarray((np.asarray(kv_w, f) * gkv[:, None]).T)

    wp_host = np.ascontiguousarray(np.asarray(proj_w, f).T)
    bp_host = np.ascontiguousarray(np.asarray(proj_b, f)[None, :])

    in_maps = []
    for core in range(8):
        b, hp = core // 2, core % 2
        X4 = x[b].reshape(C, N_TOK)
        Y4 = y[b].reshape(C, N_TOK)
        xq = np.ascontiguousarray(
            X4.reshape(C, 16, 256)[:, :, hp * 128 : (hp + 1) * 128]).reshape(C, 2048)
        Y8 = Y4.reshape(C, 8, 512)
        yk = np.ascontiguousarray(
            Y8[:, :, hp * 128 : (hp + 1) * 128]).reshape(C, 1024)
        yv = np.ascontiguousarray(
            Y8[:, :, 256 + hp * 128 : 256 + (hp + 1) * 128]).reshape(C, 1024)
        in_maps.append({
            "xq": xq, "yk": yk, "yv": yv,
            "wq": wq_host, "bq": bq_full[None, :],
            "wkv": wkv_host, "bkv": bkv_full[None, :],
            "wp": wp_host, "bp": bp_host,
            "onesd": np.ones((1, 1040), np.float32),
        })
    return in_maps


def _get_nc():
    if "nc" not in _CACHE:
        _CACHE["nc"] = _build()
    return _CACHE["nc"]


def kernel(x, y, H=64, W=64, q_w=None, q_gamma=None, q_beta=None, q_mean=None,
           q_var=None, kv_w=None, kv_gamma=None, kv_beta=None, kv_mean=None,
           kv_var=None, proj_w=None, proj_b=None, _trace=False):
    from concourse.bass_utils import run_bass_kernel_spmd

    nc = _get_nc()
    in_maps = _prep_inputs(x, y, q_w, q_gamma, q_beta, q_mean, q_var,
                           kv_w, kv_gamma, kv_beta, kv_mean, kv_var,
                           proj_w, proj_b)
    kw = {}
    if _trace:
        kw = {"trace": True, "trace_cores": list(range(8))}
    res = run_bass_kernel_spmd(nc, in_maps, list(range(8)), **kw)
    out = np.empty((B, N_TOK, C), np.float32)
    for core in range(8):
        b, hp = core // 2, core % 2
        out[b, hp * 2048 : (hp + 1) * 2048, :] = res.results[core]["out"]
    if _trace:
        return out, res
    return out


# revision 13
# speedup vs baseline: 1.0142x; 1.0142x over previous
"""Trainium2 Bass kernel for nn_Attention_57423712748130.

Computation (per batch b):
  X4 = x[b] viewed (C=256, N=4096)   [raw reshape]
  Q4 = silu(BN(q_w @ X4))            (256, 4096)
  KV4 = silu(BN(kv_w @ Y4))          (128, 4096)
  q[n,h,d]  = Q4[n1, n0*256+h*64+d]      n = n1*16+n0
  k[m,h,d]  = KV4[m1, m0*512 + h*64+d]   m = m1*8+m0
  v[m,h,d]  = KV4[m1, m0*512+256+h*64+d]
  att = softmax(q k^T / 8); o = att v
  out rows [h*1024,(h+1)*1024) = O_h @ proj_w.T + proj_b
    where O_h[n2, n3*64+d] = o[4*n2+n3, d]

Sharding: 8 cores = (batch b in 0..3) x (head-pair hp in 0..1); each core
computes heads {2hp, 2hp+1} of batch b = rows [hp*2048,(hp+1)*2048) of out[b].

On-core strategy:
 - conv outputs are computed directly in transposed layout (x/y tiles as the
   matmul stationary), so q^T/k^T slices ([d on partitions]) need no transposes
 - silu computed as x*(1+tanh(x/2)) = 2*silu(x) — tanh shares the ACT table
   set with exp (plain Silu would thrash ACT_TABLE_LOADs against Exp); the
   2x is folded: exp scale 1/32, and the V/ones side also 2x so
   normalization cancels everything
 - scoresT[m,n] = k^T.T @ q^T in PSUM, one [128,1024] exp per psum pair
 - att@v contracts over m with an extra ones-column on V producing the
   softmax denominators as row 64 of the output; reciprocals are batched
   into one [16,256] DVE op per head (iterative divide is ~7ns/elem/lane —
   per-n0 [1,256] calls would serialize 56us of it)
 - normalization grids are broadcast via a DRAM bounce, O_h columns are kept
   in a (q0-major) permuted order so every PSUM evacuation is contiguous;
   the final DMA un-permutes rows into the output
 - all matmuls in float32r (TF32-ish): 2 cyc/col stream, ~1.6e-4 matmul rel
   error vs 2.3e-3 for bf16
"""

import numpy as np

B = 4
N_TOK = 4096
C = 256
BN_EPS = 1e-5

_CACHE = {}


def _build():
    import concourse.bacc as bacc
    import concourse.bass as bass
    import concourse.tile as tile
    from concourse import mybir

    f32 = mybir.dt.float32
    f32r = mybir.dt.float32r
    AF = mybir.ActivationFunctionType

    nc = bacc.Bacc("TRN2", target_bir_lowering=False, debug=False, num_devices=8)

    xq = nc.dram_tensor("xq", [256, 2048], f32, kind="ExternalInput")
    yk = nc.dram_tensor("yk", [256, 1024], f32, kind="ExternalInput")
    yv = nc.dram_tensor("yv", [256, 1024], f32, kind="ExternalInput")
    wq = nc.dram_tensor("wq", [256, 256], f32, kind="ExternalInput")
    bq = nc.dram_tensor("bq", [1, 512], f32, kind="ExternalInput")
    wkv = nc.dram_tensor("wkv", [256, 128], f32, kind="ExternalInput")
    bkv = nc.dram_tensor("bkv", [1, 256], f32, kind="ExternalInput")
    wp = nc.dram_tensor("wp", [256, 256], f32, kind="ExternalInput")
    bp = nc.dram_tensor("bp", [1, 512], f32, kind="ExternalInput")
    onesd = nc.dram_tensor("onesd", [1, 512], f32, kind="ExternalInput")
    twod = nc.dram_tensor("twod", [1, 1040], f32, kind="ExternalInput")
    out = nc.dram_tensor("out", [2048, 256], f32, kind="ExternalOutput")
    rscr = nc.dram_tensor("rscr", [2, 4096], f32)
    rraw = nc.dram_tensor("rraw", [2, 4096], f32)

    with tile.TileContext(nc) as tc:
        with (
            tc.tile_pool(name="const", bufs=1) as cp,
            tc.tile_pool(name="actt", bufs=3) as actt,
            tc.tile_pool(name="attp", bufs=6) as attp,
            tc.tile_pool(name="outp", bufs=3) as outp,
            tc.tile_pool(name="psc", bufs=2, space="PSUM") as psc,
            tc.tile_pool(name="pss", bufs=2, space="PSUM") as pss,
            tc.tile_pool(name="pso", bufs=2, space="PSUM") as pso,
        ):
            # ---- load weights / inputs ----
            def load(t_dram, shape, tag, rows=None):
                t = cp.tile(shape, f32r, tag=tag, name=tag)
                src = t_dram.ap().bitcast(f32r)
                if rows is not None:
                    src = src[rows[0] : rows[1], :]
                nc.sync.dma_start(t[:], src)
                return t

            wq_sb = [load(wq, [128, 256], f"wq{i}", (i * 128, (i + 1) * 128)) for i in range(2)]
            wkv_sb = [load(wkv, [128, 128], f"wkv{i}", (i * 128, (i + 1) * 128)) for i in range(2)]
            wp_sb = [load(wp, [128, 256], f"wp{i}", (i * 128, (i + 1) * 128)) for i in range(2)]
            bq_sb = load(bq, [1, 512], "bq")
            bkv_sb = load(bkv, [1, 256], "bkv")
            bp_sb = load(bp, [1, 512], "bp")
            ones_sb = load(onesd, [1, 512], "ones")
            xq_sb = [load(xq, [128, 2048], f"xq{i}", (i * 128, (i + 1) * 128)) for i in range(2)]
            yk_sb = [load(yk, [128, 1024], f"yk{i}", (i * 128, (i + 1) * 128)) for i in range(2)]
            yv_sb = [load(yv, [128, 1024], f"yv{i}", (i * 128, (i + 1) * 128)) for i in range(2)]

            # conv epilogue: psum holds z = W@x + b; produce 2*silu(z) =
            # z*(1+tanh(z/2)) into dst (f32r)
            def silu2(ps, dst, tag):
                t = actt.tile(list(ps.shape), f32, tag="tanh_t", name=f"t_{tag}")
                u = actt.tile(list(ps.shape), f32, tag="tanh_u", name=f"u_{tag}")
                nc.scalar.activation(t[:], ps, AF.Tanh, scale=0.5)
                nc.vector.tensor_mul(u[:], ps, t[:])
                nc.vector.tensor_add(dst, ps, u[:])

            # ---- kv conv (k part): kT[pp, m0, m1], pp = hl*64+d ----
            kT = cp.tile([128, 8, 128], f32r, tag="kT")
            for mt in range(4):  # m0 pairs
                ps = psc.tile([128, 512], f32, tag="cnv")
                for mi in range(2):
                    m0 = 2 * mt + mi
                    for c0 in range(2):
                        nc.tensor.matmul(
                            ps[:, mi * 128 : (mi + 1) * 128],
                            lhsT=yk_sb[c0][:, m0 * 128 : (m0 + 1) * 128],
                            rhs=wkv_sb[c0][:],
                            start=(c0 == 0), stop=False)
                    nc.tensor.matmul(
                        ps[:, mi * 128 : (mi + 1) * 128],
                        lhsT=ones_sb[0:1, 0:128],
                        rhs=bkv_sb[0:1, mi * 128 : (mi + 1) * 128],
                        start=False, stop=True)
                silu2(ps[:, 0:256], kT[:, 2 * mt : 2 * mt + 2, :].rearrange("p a b -> p (a b)"), f"k{mt}")

            # ---- kv conv (v part): vext[m1, m0, hl, 0:64]=2v, [...,64]=2 ----
            vext = cp.tile([128, 8, 2, 65], f32r, tag="vext")
            nc.sync.dma_start(
                vext[:], twod.ap().bitcast(f32r).partition_broadcast(128))
            for jv in range(2):
                ps = psc.tile([128, 512], f32, tag="cnv")
                for c0 in range(2):
                    nc.tensor.matmul(
                        ps[:], lhsT=wkv_sb[c0][:],
                        rhs=yv_sb[c0][:, jv * 512 : (jv + 1) * 512],
                        start=(c0 == 0), stop=False)
                nc.tensor.matmul(
                    ps[:], lhsT=bkv_sb[0:1, 0:128], rhs=ones_sb[:],
                    start=False, stop=True)
                vdst = vext[:, jv * 4 : (jv + 1) * 4, :, 0:64]
                psv = ps[:].rearrange("p (a h d) -> p a h d", a=4, h=2)
                t = actt.tile([128, 512], f32, tag="tanh_t", name=f"t_v{jv}")
                u = actt.tile([128, 512], f32, tag="tanh_u", name=f"u_v{jv}")
                nc.scalar.activation(t[:], ps[:], AF.Tanh, scale=0.5)
                nc.vector.tensor_mul(u[:], ps[:], t[:])
                nc.vector.tensor_add(
                    vdst, psv, u[:].rearrange("p (a h d) -> p a h d", a=4, h=2))

            # ---- q conv: qT[pp, n0, n1] ----
            qT = cp.tile([128, 16, 256], f32r, tag="qT")
            for t2 in range(8):  # n0 pairs
                ps = psc.tile([128, 512], f32, tag="cnv")
                for nn in range(2):
                    n0 = 2 * t2 + nn
                    for c0 in range(2):
                        nc.tensor.matmul(
                            ps[:, nn * 256 : (nn + 1) * 256],
                            lhsT=xq_sb[c0][:, n0 * 128 : (n0 + 1) * 128],
                            rhs=wq_sb[c0][:],
                            start=(c0 == 0), stop=False)
                    nc.tensor.matmul(
                        ps[:, nn * 256 : (nn + 1) * 256],
                        lhsT=ones_sb[0:1, 0:128],
                        rhs=bq_sb[0:1, nn * 256 : (nn + 1) * 256],
                        start=False, stop=True)
                silu2(ps[:], qT[:, 2 * t2 : 2 * t2 + 2, :].rearrange("p a b -> p (a b)"), f"q{t2}")

            # ---- attention ----
            for hl in range(2):
                r0, r1 = hl * 64, (hl + 1) * 64
                sums = cp.tile([16, 256], f32, tag=f"sums{hl}", name=f"sums{hl}")
                sums_row = cp.tile([1, 4096], f32, tag=f"sumsrow{hl}",
                                   name=f"sumsrow{hl}")
                outun = [
                    cp.tile([128, 1024], f32, tag=f"outun{hl}_{i}", name=f"outun{hl}_{i}")
                    for i in range(2)
                ]
                for t2 in range(8):  # n0 pairs (n0 = 2*t2 + nn)
                    att = []
                    for j in range(4):  # m0 = 2j + mi
                        scp = pss.tile([128, 1024], f32, tag="scp")
                        for mi in range(2):
                            m0 = 2 * j + mi
                            nc.tensor.matmul(
                                scp[:, mi * 512 : (mi + 1) * 512],
                                lhsT=kT[r0:r1, m0, :],
                                rhs=qT[r0:r1, 2 * t2 : 2 * t2 + 2, :],
                                start=True, stop=True)
                        a = attp.tile([128, 1024], f32r, tag="att",
                                      name=f"att{hl}_{t2}_{j}")
                        # z = 4*q.k ; softmax wants exp(q.k/8) -> scale 1/32
                        nc.scalar.activation(a[:], scp[:], AF.Exp, scale=0.03125)
                        att.append(a)
                    ops = pso.tile([65, 512], f32, tag="ops")
                    for m0 in range(8):
                        nc.tensor.matmul(
                            ops[:], lhsT=vext[:, m0, hl, :],
                            rhs=att[m0 // 2][:, (m0 % 2) * 512 : (m0 % 2 + 1) * 512],
                            start=(m0 == 0), stop=(m0 == 7))
                    nc.vector.tensor_copy(
                        sums_row[0:1, t2 * 512 : (t2 + 1) * 512], ops[64:65, :])
                    for nn in range(2):
                        n0 = 2 * t2 + nn
                        n3, q0 = n0 & 3, n0 >> 2
                        # o (x2) -> outun[c][band, q0-major cols] (contiguous)
                        dst = outun[n3 // 2][
                            (n3 % 2) * 64 : (n3 % 2) * 64 + 64,
                            q0 * 256 : (q0 + 1) * 256]
                        nc.vector.tensor_copy(
                            dst.bitcast(f32r),
                            ops[0:64, nn * 256 : (nn + 1) * 256])

                nc.sync.dma_start(
                    bass.AP(tensor=rraw, offset=hl * 4096,
                            ap=[[4096, 1], [1, 4096]]),
                    sums_row[:])
                # gather raw sums (n0-major) into [16,256] tile in
                # (p=n3*4+q0, n1) order: 4 affine DMAs (one per n3)
                for n3g in range(4):
                    nc.sync.dma_start(
                        sums[n3g * 4 : (n3g + 1) * 4, :],
                        bass.AP(tensor=rraw, offset=hl * 4096 + n3g * 256,
                                ap=[[1024, 4], [1, 256]]))
                # 1/(2S) for all 4096 tokens in one shot
                rec = cp.tile([16, 256], f32, tag=f"rec{hl}", name=f"rec{hl}")
                nc.vector.reciprocal(rec[:], sums[:])
                nc.sync.dma_start(
                    bass.AP(tensor=rscr, offset=hl * 4096,
                            ap=[[256, 16], [1, 256]]),
                    rec[:])
                for c0 in range(2):
                    grid = cp.tile([128, 1024], f32, tag=f"grid{hl}_{c0}",
                                   name=f"grid{hl}_{c0}")
                    for bnd in range(2):
                        n3g = c0 * 2 + bnd
                        src = bass.AP(
                            tensor=rscr, offset=hl * 4096 + n3g * 1024,
                            ap=[[0, 64], [1, 1024]])
                        nc.sync.dma_start(grid[bnd * 64 : (bnd + 1) * 64, :], src)
                    nc.vector.tensor_mul(
                        outun[c0][:].bitcast(f32r), outun[c0][:], grid[:])

                # ---- proj: fc' pairs; un-permute rows in the output DMA ----
                for fp in range(4):
                    ps = psc.tile([128, 512], f32, tag="cnv")
                    for half in range(2):
                        fc = 2 * fp + half
                        for c0 in range(2):
                            nc.tensor.matmul(
                                ps[:, half * 256 : (half + 1) * 256],
                                lhsT=outun[c0][:].bitcast(f32r)[
                                    :, fc * 128 : (fc + 1) * 128],
                                rhs=wp_sb[c0][:],
                                start=(c0 == 0), stop=False)
                        nc.tensor.matmul(
                            ps[:, half * 256 : (half + 1) * 256],
                            lhsT=ones_sb[0:1, 0:128],
                            rhs=bp_sb[0:1, half * 256 : (half + 1) * 256],
                            start=False, stop=True)
                    osb = outp.tile([128, 512], f32, tag="osb",
                                    name=f"osb{hl}_{fp}")
                    nc.vector.tensor_copy(osb[:], ps[:])
                    # fc = 2*fp+half -> q0 = fp, rows hl*1024 + half*512 + fp + 4r
                    dstap = bass.AP(
                        tensor=out,
                        offset=(hl * 1024 + fp) * 256,
                        ap=[[4 * 256, 128], [512 * 256, 2], [1, 256]])
                    nc.sync.dma_start(
                        dstap,
                        osb[:].rearrange("p (h c) -> p h c", h=2))

    nc.compile()
    return nc


def _prep_inputs(x, y, q_w, q_gamma, q_beta, q_mean, q_var,
                 kv_w, kv_gamma, kv_beta, kv_mean, kv_var, proj_w, proj_b):
    f = np.float32
    x = np.ascontiguousarray(np.asarray(x, f))
    y = np.ascontiguousarray(np.asarray(y, f))

    gq = np.asarray(q_gamma, f) / np.sqrt(np.asarray(q_var, f) + BN_EPS)
    bq_full = np.asarray(q_beta, f) - np.asarray(q_mean, f) * gq
    wq_host = np.ascontiguousarray((np.asarray(q_w, f) * gq[:, None]).T)

    gkv = np.asarray(kv_gamma, f) / np.sqrt(np.asarray(kv_var, f) + BN_EPS)
    bkv_full = np.asarray(kv_beta, f) - np.asarray(kv_mean, f) * gkv
    wkv_host = np.ascontiguousarray((np.asarray(kv_w, f) * gkv[:, None]).T)

    wp_host = np.ascontiguousarray(np.asarray(proj_w, f).T)
    bp_host = np.asarray(proj_b, f)

    bq2 = np.tile(bq_full[None, :], (1, 2)).astype(f)
    bkv2 = np.tile(bkv_full[None, :], (1, 2)).astype(f)
    bp2 = np.tile(bp_host[None, :], (1, 2)).astype(f)

    in_maps = []
    for core in range(8):
        b, hp = core // 2, core % 2
        X4 = x[b].reshape(C, N_TOK)
        Y4 = y[b].reshape(C, N_TOK)
        xqa = np.ascontiguousarray(
            X4.reshape(C, 16, 256)[:, :, hp * 128 : (hp + 1) * 128]).reshape(C, 2048)
        Y8 = Y4.reshape(C, 8, 512)
        yka = np.ascontiguousarray(
            Y8[:, :, hp * 128 : (hp + 1) * 128]).reshape(C, 1024)
        yva = np.ascontiguousarray(
            Y8[:, :, 256 + hp * 128 : 256 + (hp + 1) * 128]).reshape(C, 1024)
        in_maps.append({
            "xq": xqa, "yk": yka, "yv": yva,
            "wq": wq_host, "bq": bq2,
            "wkv": wkv_host, "bkv": bkv2,
            "wp": wp_host, "bp": bp2,
            "onesd": np.ones((1, 512), f),
            "twod": np.full((1, 1040), 2.0, f),
        })
    return in_maps


def _get_nc():
    if "nc" not in _CACHE:
        _CACHE["nc"] = _build()
    return _CACHE["nc"]


def kernel(x, y, H=64, W=64, q_w=None, q_gamma=None, q_beta=None, q_mean=None,
           q_var=None, kv_w=None, kv_gamma=None, kv_beta=None, kv_mean=None,
           kv_var=None, proj_w=None, proj_b=None, _trace=False):
    from concourse.bass_utils import run_bass_kernel_spmd

    nc = _get_nc()
    in_maps = _prep_inputs(x, y, q_w, q_gamma, q_beta, q_mean, q_var,
                           kv_w, kv_gamma, kv_beta, kv_mean, kv_var,
                           proj_w, proj_b)
    kw = {}
    if _trace:
        kw = {"trace": True, "trace_cores": list(range(8))}
    res = run_bass_kernel_spmd(nc, in_maps, list(range(8)), **kw)
    outa = np.empty((B, N_TOK, C), np.float32)
    for core in range(8):
        b, hp = core // 2, core % 2
        outa[b, hp * 2048 : (hp + 1) * 2048, :] = res.results[core]["out"]
    if _trace:
        return outa, res
    return outa


# revision 14
# speedup vs baseline: 1.2307x; 1.2134x over previous
"""Trainium2 Bass kernel for nn_Attention_57423712748130.

Computation (per batch b):
  X4 = x[b] viewed (C=256, N=4096)   [raw reshape]
  Q4 = silu(BN(q_w @ X4))            (256, 4096)
  KV4 = silu(BN(kv_w @ Y4))          (128, 4096)
  q[n,h,d]  = Q4[n1, n0*256+h*64+d]      n = n1*16+n0
  k[m,h,d]  = KV4[m1, m0*512 + h*64+d]   m = m1*8+m0
  v[m,h,d]  = KV4[m1, m0*512+256+h*64+d]
  att = softmax(q k^T / 8); o = att v
  out rows [h*1024,(h+1)*1024) = O_h @ proj_w.T + proj_b
    where O_h[n2, n3*64+d] = o[4*n2+n3, d]

Sharding: 8 cores = (batch b in 0..3) x (head-pair hp in 0..1); each core
computes heads {2hp, 2hp+1} of batch b = rows [hp*2048,(hp+1)*2048) of out[b].

On-core strategy:
 - conv outputs are computed directly in transposed layout (x/y tiles as the
   matmul stationary), so q^T/k^T slices ([d on partitions]) need no transposes
 - silu computed as x*(1+tanh(x/2)) = 2*silu(x) — tanh shares the ACT table
   set with exp (plain Silu would thrash ACT_TABLE_LOADs against Exp); the
   2x is folded: exp scale 1/32, and the V/ones side also 2x so
   normalization cancels everything
 - scoresT[m,n] = k^T.T @ q^T in PSUM, one [128,1024] exp per psum pair
 - att@v contracts over m with an extra ones-column on V producing the
   softmax denominators as row 64 of the output; reciprocals are batched
   into one [16,256] DVE op per head (iterative divide is ~7ns/elem/lane —
   per-n0 [1,256] calls would serialize 56us of it)
 - normalization grids are broadcast via a DRAM bounce, O_h columns are kept
   in a (q0-major) permuted order so every PSUM evacuation is contiguous;
   the final DMA un-permutes rows into the output
 - all matmuls in float32r (TF32-ish): 2 cyc/col stream, ~1.6e-4 matmul rel
   error vs 2.3e-3 for bf16
"""

import ml_dtypes
import numpy as np

B = 4
N_TOK = 4096
C = 256
BN_EPS = 1e-5
ATTN_BF16 = True  # scores/att@v operands in bf16 (PSUM accum stays fp32)

_CACHE = {}


def _build():
    import concourse.bacc as bacc
    import concourse.bass as bass
    import concourse.tile as tile
    from concourse import mybir

    f32 = mybir.dt.float32
    f32r = mybir.dt.float32r
    bf16 = mybir.dt.bfloat16
    adt = bf16 if ATTN_BF16 else f32r
    AF = mybir.ActivationFunctionType

    nc = bacc.Bacc("TRN2", target_bir_lowering=False, debug=False, num_devices=8)

    xq = nc.dram_tensor("xq", [256, 2048], f32, kind="ExternalInput")
    yk = nc.dram_tensor("yk", [256, 1024], f32, kind="ExternalInput")
    yv = nc.dram_tensor("yv", [256, 1024], f32, kind="ExternalInput")
    wq = nc.dram_tensor("wq", [256, 256], f32, kind="ExternalInput")
    bq = nc.dram_tensor("bq", [1, 512], f32, kind="ExternalInput")
    wkv = nc.dram_tensor("wkv", [256, 128], f32, kind="ExternalInput")
    bkv = nc.dram_tensor("bkv", [1, 256], f32, kind="ExternalInput")
    wp = nc.dram_tensor("wp", [256, 256], f32, kind="ExternalInput")
    bp = nc.dram_tensor("bp", [1, 512], f32, kind="ExternalInput")
    onesd = nc.dram_tensor("onesd", [1, 512], f32, kind="ExternalInput")
    twod = nc.dram_tensor("twod", [1, 1040], adt, kind="ExternalInput")
    out = nc.dram_tensor("out", [2048, 256], f32, kind="ExternalOutput")
    rscr = nc.dram_tensor("rscr", [2, 4096], f32)
    rraw = nc.dram_tensor("rraw", [2, 4096], f32)

    with tile.TileContext(nc) as tc:
        with (
            tc.tile_pool(name="const", bufs=1) as cp,
            tc.tile_pool(name="actt", bufs=3) as actt,
            tc.tile_pool(name="attp", bufs=6) as attp,
            tc.tile_pool(name="outp", bufs=3) as outp,
            tc.tile_pool(name="psc", bufs=2, space="PSUM") as psc,
            tc.tile_pool(name="pss", bufs=2, space="PSUM") as pss,
            tc.tile_pool(name="pso", bufs=2, space="PSUM") as pso,
        ):
            # ---- load weights / inputs ----
            def load(t_dram, shape, tag, rows=None):
                t = cp.tile(shape, f32r, tag=tag, name=tag)
                src = t_dram.ap().bitcast(f32r)
                if rows is not None:
                    src = src[rows[0] : rows[1], :]
                nc.sync.dma_start(t[:], src)
                return t

            wq_sb = [load(wq, [128, 256], f"wq{i}", (i * 128, (i + 1) * 128)) for i in range(2)]
            wkv_sb = [load(wkv, [128, 128], f"wkv{i}", (i * 128, (i + 1) * 128)) for i in range(2)]
            wp_sb = [load(wp, [128, 256], f"wp{i}", (i * 128, (i + 1) * 128)) for i in range(2)]
            bq_sb = load(bq, [1, 512], "bq")
            bkv_sb = load(bkv, [1, 256], "bkv")
            bp_sb = load(bp, [1, 512], "bp")
            ones_sb = load(onesd, [1, 512], "ones")
            xq_sb = [load(xq, [128, 2048], f"xq{i}", (i * 128, (i + 1) * 128)) for i in range(2)]
            yk_sb = [load(yk, [128, 1024], f"yk{i}", (i * 128, (i + 1) * 128)) for i in range(2)]
            yv_sb = [load(yv, [128, 1024], f"yv{i}", (i * 128, (i + 1) * 128)) for i in range(2)]

            # conv epilogue: psum holds z = W@x + b; produce 2*silu(z) =
            # z*(1+tanh(z/2)) into dst (f32r)
            def silu2(ps, dst, tag):
                t = actt.tile(list(ps.shape), f32, tag="tanh_t", name=f"t_{tag}")
                u = actt.tile(list(ps.shape), f32, tag="tanh_u", name=f"u_{tag}")
                nc.scalar.activation(t[:], ps, AF.Tanh, scale=0.5)
                nc.vector.tensor_mul(u[:], ps, t[:])
                nc.vector.tensor_add(dst, ps, u[:])

            # ---- kv conv (k part): kT[pp, m0, m1], pp = hl*64+d ----
            kT = cp.tile([128, 8, 128], adt, tag="kT")
            for mt in range(4):  # m0 pairs
                ps = psc.tile([128, 512], f32, tag="cnv")
                for mi in range(2):
                    m0 = 2 * mt + mi
                    for c0 in range(2):
                        nc.tensor.matmul(
                            ps[:, mi * 128 : (mi + 1) * 128],
                            lhsT=yk_sb[c0][:, m0 * 128 : (m0 + 1) * 128],
                            rhs=wkv_sb[c0][:],
                            start=(c0 == 0), stop=False)
                    nc.tensor.matmul(
                        ps[:, mi * 128 : (mi + 1) * 128],
                        lhsT=ones_sb[0:1, 0:128],
                        rhs=bkv_sb[0:1, mi * 128 : (mi + 1) * 128],
                        start=False, stop=True)
                silu2(ps[:, 0:256], kT[:, 2 * mt : 2 * mt + 2, :].rearrange("p a b -> p (a b)"), f"k{mt}")

            # ---- kv conv (v part): vext[m1, m0, hl, 0:64]=2v, [...,64]=2 ----
            vext = cp.tile([128, 8, 2, 65], adt, tag="vext")
            nc.sync.dma_start(
                vext[:], twod.ap().partition_broadcast(128))
            for jv in range(2):
                ps = psc.tile([128, 512], f32, tag="cnv")
                for c0 in range(2):
                    nc.tensor.matmul(
                        ps[:], lhsT=wkv_sb[c0][:],
                        rhs=yv_sb[c0][:, jv * 512 : (jv + 1) * 512],
                        start=(c0 == 0), stop=False)
                nc.tensor.matmul(
                    ps[:], lhsT=bkv_sb[0:1, 0:128], rhs=ones_sb[:],
                    start=False, stop=True)
                vdst = vext[:, jv * 4 : (jv + 1) * 4, :, 0:64]
                psv = ps[:].rearrange("p (a h d) -> p a h d", a=4, h=2)
                t = actt.tile([128, 512], f32, tag="tanh_t", name=f"t_v{jv}")
                u = actt.tile([128, 512], f32, tag="tanh_u", name=f"u_v{jv}")
                nc.scalar.activation(t[:], ps[:], AF.Tanh, scale=0.5)
                nc.vector.tensor_mul(u[:], ps[:], t[:])
                nc.vector.tensor_add(
                    vdst, psv, u[:].rearrange("p (a h d) -> p a h d", a=4, h=2))

            # ---- q conv: qT[pp, n0, n1] ----
            qT = cp.tile([128, 16, 256], adt, tag="qT")
            for t2 in range(8):  # n0 pairs
                ps = psc.tile([128, 512], f32, tag="cnv")
                for nn in range(2):
                    n0 = 2 * t2 + nn
                    for c0 in range(2):
                        nc.tensor.matmul(
                            ps[:, nn * 256 : (nn + 1) * 256],
                            lhsT=xq_sb[c0][:, n0 * 128 : (n0 + 1) * 128],
                            rhs=wq_sb[c0][:],
                            start=(c0 == 0), stop=False)
                    nc.tensor.matmul(
                        ps[:, nn * 256 : (nn + 1) * 256],
                        lhsT=ones_sb[0:1, 0:128],
                        rhs=bq_sb[0:1, nn * 256 : (nn + 1) * 256],
                        start=False, stop=True)
                silu2(ps[:], qT[:, 2 * t2 : 2 * t2 + 2, :].rearrange("p a b -> p (a b)"), f"q{t2}")

            # ---- attention ----
            for hl in range(2):
                r0, r1 = hl * 64, (hl + 1) * 64
                sums = cp.tile([16, 256], f32, tag=f"sums{hl}", name=f"sums{hl}")
                sums_row = cp.tile([1, 4096], f32, tag=f"sumsrow{hl}",
                                   name=f"sumsrow{hl}")
                outun = [
                    cp.tile([128, 1024], f32, tag=f"outun{hl}_{i}", name=f"outun{hl}_{i}")
                    for i in range(2)
                ]
                for t2 in range(8):  # n0 pairs (n0 = 2*t2 + nn)
                    att = []
                    for j in range(4):  # m0 = 2j + mi
                        scp = pss.tile([128, 1024], f32, tag="scp")
                        for mi in range(2):
                            m0 = 2 * j + mi
                            nc.tensor.matmul(
                                scp[:, mi * 512 : (mi + 1) * 512],
                                lhsT=kT[r0:r1, m0, :],
                                rhs=qT[r0:r1, 2 * t2 : 2 * t2 + 2, :],
                                start=True, stop=True)
                        a = attp.tile([128, 1024], adt, tag="att",
                                      name=f"att{hl}_{t2}_{j}")
                        # z = 4*q.k ; softmax wants exp(q.k/8) -> scale 1/32
                        nc.scalar.activation(a[:], scp[:], AF.Exp, scale=0.03125)
                        att.append(a)
                    ops = pso.tile([65, 512], f32, tag="ops")
                    for m0 in range(8):
                        nc.tensor.matmul(
                            ops[:], lhsT=vext[:, m0, hl, :],
                            rhs=att[m0 // 2][:, (m0 % 2) * 512 : (m0 % 2 + 1) * 512],
                            start=(m0 == 0), stop=(m0 == 7))
                    nc.vector.tensor_copy(
                        sums_row[0:1, t2 * 512 : (t2 + 1) * 512], ops[64:65, :])
                    for nn in range(2):
                        n0 = 2 * t2 + nn
                        n3, q0 = n0 & 3, n0 >> 2
                        # o (x2) -> outun[c][band, q0-major cols] (contiguous)
                        dst = outun[n3 // 2][
                            (n3 % 2) * 64 : (n3 % 2) * 64 + 64,
                            q0 * 256 : (q0 + 1) * 256]
                        nc.vector.tensor_copy(
                            dst.bitcast(f32r),
                            ops[0:64, nn * 256 : (nn + 1) * 256])

                nc.sync.dma_start(
                    bass.AP(tensor=rraw, offset=hl * 4096,
                            ap=[[4096, 1], [1, 4096]]),
                    sums_row[:])
                # gather raw sums (n0-major) into [16,256] tile in
                # (p=n3*4+q0, n1) order: 4 affine DMAs (one per n3)
                for n3g in range(4):
                    nc.sync.dma_start(
                        sums[n3g * 4 : (n3g + 1) * 4, :],
                        bass.AP(tensor=rraw, offset=hl * 4096 + n3g * 256,
                                ap=[[1024, 4], [1, 256]]))
                # 1/(2S) for all 4096 tokens in one shot
                rec = cp.tile([16, 256], f32, tag=f"rec{hl}", name=f"rec{hl}")
                nc.vector.reciprocal(rec[:], sums[:])
                nc.sync.dma_start(
                    bass.AP(tensor=rscr, offset=hl * 4096,
                            ap=[[256, 16], [1, 256]]),
                    rec[:])
                for c0 in range(2):
                    grid = cp.tile([128, 1024], f32, tag=f"grid{hl}_{c0}",
                                   name=f"grid{hl}_{c0}")
                    for bnd in range(2):
                        n3g = c0 * 2 + bnd
                        src = bass.AP(
                            tensor=rscr, offset=hl * 4096 + n3g * 1024,
                            ap=[[0, 64], [1, 1024]])
                        nc.sync.dma_start(grid[bnd * 64 : (bnd + 1) * 64, :], src)
                    nc.vector.tensor_mul(
                        outun[c0][:].bitcast(f32r), outun[c0][:], grid[:])

                # ---- proj: fc' pairs; un-permute rows in the output DMA ----
                for fp in range(4):
                    ps = psc.tile([128, 512], f32, tag="cnv")
                    for half in range(2):
                        fc = 2 * fp + half
                        for c0 in range(2):
                            nc.tensor.matmul(
                                ps[:, half * 256 : (half + 1) * 256],
                                lhsT=outun[c0][:].bitcast(f32r)[
                                    :, fc * 128 : (fc + 1) * 128],
                                rhs=wp_sb[c0][:],
                                start=(c0 == 0), stop=False)
                        nc.tensor.matmul(
                            ps[:, half * 256 : (half + 1) * 256],
                            lhsT=ones_sb[0:1, 0:128],
                            rhs=bp_sb[0:1, half * 256 : (half + 1) * 256],
                            start=False, stop=True)
                    osb = outp.tile([128, 512], f32, tag="osb",
                                    name=f"osb{hl}_{fp}")
                    nc.vector.tensor_copy(osb[:], ps[:])
                    # fc = 2*fp+half -> q0 = fp, rows hl*1024 + half*512 + fp + 4r
                    dstap = bass.AP(
                        tensor=out,
                        offset=(hl * 1024 + fp) * 256,
                        ap=[[4 * 256, 128], [512 * 256, 2], [1, 256]])
                    nc.sync.dma_start(
                        dstap,
                        osb[:].rearrange("p (h c) -> p h c", h=2))

    nc.compile()
    return nc


def _prep_inputs(x, y, q_w, q_gamma, q_beta, q_mean, q_var,
                 kv_w, kv_gamma, kv_beta, kv_mean, kv_var, proj_w, proj_b):
    f = np.float32
    x = np.ascontiguousarray(np.asarray(x, f))
    y = np.ascontiguousarray(np.asarray(y, f))

    gq = np.asarray(q_gamma, f) / np.sqrt(np.asarray(q_var, f) + BN_EPS)
    bq_full = np.asarray(q_beta, f) - np.asarray(q_mean, f) * gq
    wq_host = np.ascontiguousarray((np.asarray(q_w, f) * gq[:, None]).T)

    gkv = np.asarray(kv_gamma, f) / np.sqrt(np.asarray(kv_var, f) + BN_EPS)
    bkv_full = np.asarray(kv_beta, f) - np.asarray(kv_mean, f) * gkv
    wkv_host = np.ascontiguousarray((np.asarray(kv_w, f) * gkv[:, None]).T)

    wp_host = np.ascontiguousarray(np.asarray(proj_w, f).T)
    bp_host = np.asarray(proj_b, f)

    bq2 = np.tile(bq_full[None, :], (1, 2)).astype(f)
    bkv2 = np.tile(bkv_full[None, :], (1, 2)).astype(f)
    bp2 = np.tile(bp_host[None, :], (1, 2)).astype(f)

    in_maps = []
    for core in range(8):
        b, hp = core // 2, core % 2
        X4 = x[b].reshape(C, N_TOK)
        Y4 = y[b].reshape(C, N_TOK)
        xqa = np.ascontiguousarray(
            X4.reshape(C, 16, 256)[:, :, hp * 128 : (hp + 1) * 128]).reshape(C, 2048)
        Y8 = Y4.reshape(C, 8, 512)
        yka = np.ascontiguousarray(
            Y8[:, :, hp * 128 : (hp + 1) * 128]).reshape(C, 1024)
        yva = np.ascontiguousarray(
            Y8[:, :, 256 + hp * 128 : 256 + (hp + 1) * 128]).reshape(C, 1024)
        in_maps.append({
            "xq": xqa, "yk": yka, "yv": yva,
            "wq": wq_host, "bq": bq2,
            "wkv": wkv_host, "bkv": bkv2,
            "wp": wp_host, "bp": bp2,
            "onesd": np.ones((1, 512), f),
            "twod": np.full((1, 1040), 2.0,
                            ml_dtypes.bfloat16 if ATTN_BF16 else f),
        })
    return in_maps


def _get_nc():
    if "nc" not in _CACHE:
        _CACHE["nc"] = _build()
    return _CACHE["nc"]


def kernel(x, y, H=64, W=64, q_w=None, q_gamma=None, q_beta=None, q_mean=None,
           q_var=None, kv_w=None, kv_gamma=None, kv_beta=None, kv_mean=None,
           kv_var=None, proj_w=None, proj_b=None, _trace=False):
    from concourse.bass_utils import run_bass_kernel_spmd

    nc = _get_nc()
    in_maps = _prep_inputs(x, y, q_w, q_gamma, q_beta, q_mean, q_var,
                           kv_w, kv_gamma, kv_beta, kv_mean, kv_var,
                           proj_w, proj_b)
    kw = {}
    if _trace:
        kw = {"trace": True, "trace_cores": list(range(8))}
    res = run_bass_kernel_spmd(nc, in_maps, list(range(8)), **kw)
    outa = np.empty((B, N_TOK, C), np.float32)
    for core in range(8):
        b, hp = core // 2, core % 2
        outa[b, hp * 2048 : (hp + 1) * 2048, :] = res.results[core]["out"]
    if _trace:
        return outa, res
    return outa


# revision 16
# speedup vs baseline: 1.4441x; 1.1734x over previous
"""Trainium2 Bass kernel for nn_Attention_57423712748130.

Computation (per batch b):
  X4 = x[b] viewed (C=256, N=4096)   [raw reshape]
  Q4 = silu(BN(q_w @ X4))            (256, 4096)
  KV4 = silu(BN(kv_w @ Y4))          (128, 4096)
  q[n,h,d]  = Q4[n1, n0*256+h*64+d]      n = n1*16+n0
  k[m,h,d]  = KV4[m1, m0*512 + h*64+d]   m = m1*8+m0
  v[m,h,d]  = KV4[m1, m0*512+256+h*64+d]
  att = softmax(q k^T / 8); o = att v
  out rows [h*1024,(h+1)*1024) = O_h @ proj_w.T + proj_b
    where O_h[n2, n3*64+d] = o[4*n2+n3, d]

Sharding: 8 cores = (batch b in 0..3) x (head-pair hp in 0..1); each core
computes heads {2hp, 2hp+1} of batch b = rows [hp*2048,(hp+1)*2048) of out[b].

On-core strategy:
 - conv outputs are computed directly in transposed layout (x/y tiles as the
   matmul stationary), so q^T/k^T slices ([d on partitions]) need no transposes
 - conv bias is applied on VectorE (a K=1 bias matmul costs ~500ns of PE each);
   silu is computed as z*(1+tanh(z/2)) = 2*silu(z) — tanh shares the ACT table
   set with exp (plain Silu thrashes ACT_TABLE_LOADs against Exp); the 2x is
   folded: exp scale 1/32 and a 2.0 fill for the V/ones column cancel it
 - scoresT[m,n] = k^T.T @ q^T in PSUM, one [128,1024] exp per 2-bank psum
 - att@v contracts over m with an extra ones-column on V producing the softmax
   denominators as row 64; reciprocal runs as one [16,64] DVE op per quarter
   (a per-n0 [1,256] reciprocal is 1.75us of iterative divide each)
 - scores/att@v operands are bf16 (PSUM accumulation fp32); convs/proj f32r.
   measured end-to-end absmax-rel ~3e-3, resid_var ~1.5e-6
 - normalization + projection run per q0-quarter, pipelined behind the
   attention of later quarters (grids broadcast via a DRAM bounce); O_h
   columns are kept q0-major so every PSUM evacuation is contiguous; the
   final output DMA un-permutes rows
"""

import ml_dtypes
import numpy as np

B = 4
N_TOK = 4096
C = 256
BN_EPS = 1e-5
ATTN_BF16 = True  # scores/att@v operands in bf16 (PSUM accum stays fp32)

_CACHE = {}


def _build():
    import concourse.bacc as bacc
    import concourse.bass as bass
    import concourse.tile as tile
    from concourse import mybir

    f32 = mybir.dt.float32
    f32r = mybir.dt.float32r
    bf16 = mybir.dt.bfloat16
    adt = bf16 if ATTN_BF16 else f32r
    AF = mybir.ActivationFunctionType

    nc = bacc.Bacc("TRN2", target_bir_lowering=False, debug=False, num_devices=8)

    xq = nc.dram_tensor("xq", [256, 2048], f32, kind="ExternalInput")
    yk = nc.dram_tensor("yk", [256, 1024], f32, kind="ExternalInput")
    yv = nc.dram_tensor("yv", [256, 1024], f32, kind="ExternalInput")
    wq = nc.dram_tensor("wq", [256, 256], f32, kind="ExternalInput")
    bq = nc.dram_tensor("bq", [1, 512], f32, kind="ExternalInput")
    wkv = nc.dram_tensor("wkv", [256, 128], f32, kind="ExternalInput")
    bkv = nc.dram_tensor("bkv", [1, 256], f32, kind="ExternalInput")
    bkvc = nc.dram_tensor("bkvc", [128, 1], f32, kind="ExternalInput")
    wp = nc.dram_tensor("wp", [256, 256], f32, kind="ExternalInput")
    bp = nc.dram_tensor("bp", [1, 512], f32, kind="ExternalInput")
    twod = nc.dram_tensor("twod", [1, 1040], adt, kind="ExternalInput")
    out = nc.dram_tensor("out", [2048, 256], f32, kind="ExternalOutput")
    rscr = nc.dram_tensor("rscr", [2, 4096], f32)
    rraw = nc.dram_tensor("rraw", [2, 4096], f32)

    with tile.TileContext(nc) as tc:
        with (
            tc.tile_pool(name="const", bufs=1) as cp,
            tc.tile_pool(name="actt", bufs=3) as actt,
            tc.tile_pool(name="attp", bufs=6) as attp,
            tc.tile_pool(name="outp", bufs=3) as outp,
            tc.tile_pool(name="nrm", bufs=3) as nrm,
            tc.tile_pool(name="psc", bufs=2, space="PSUM") as psc,
            tc.tile_pool(name="pss", bufs=2, space="PSUM") as pss,
            tc.tile_pool(name="pso", bufs=2, space="PSUM") as pso,
        ):
            # ---- load weights / inputs ----
            def load(t_dram, shape, tag, rows=None):
                t = cp.tile(shape, f32r, tag=tag, name=tag)
                src = t_dram.ap().bitcast(f32r)
                if rows is not None:
                    src = src[rows[0] : rows[1], :]
                nc.sync.dma_start(t[:], src)
                return t

            def load_bcast(t_dram, shape, tag):
                t = cp.tile(shape, f32, tag=tag, name=tag)
                nc.sync.dma_start(t[:], t_dram.ap().partition_broadcast(shape[0]))
                return t

            wq_sb = [load(wq, [128, 256], f"wq{i}", (i * 128, (i + 1) * 128)) for i in range(2)]
            wkv_sb = [load(wkv, [128, 128], f"wkv{i}", (i * 128, (i + 1) * 128)) for i in range(2)]
            wp_sb = [load(wp, [128, 256], f"wp{i}", (i * 128, (i + 1) * 128)) for i in range(2)]
            bq_bc = load_bcast(bq, [128, 512], "bq_bc")
            bkv_bc = load_bcast(bkv, [128, 256], "bkv_bc")
            bp_bc = load_bcast(bp, [128, 512], "bp_bc")
            bkvc_sb = cp.tile([128, 1], f32, tag="bkvc", name="bkvc")
            nc.sync.dma_start(bkvc_sb[:], bkvc.ap())
            xq_sb = [load(xq, [128, 2048], f"xq{i}", (i * 128, (i + 1) * 128)) for i in range(2)]
            yk_sb = [load(yk, [128, 1024], f"yk{i}", (i * 128, (i + 1) * 128)) for i in range(2)]
            yv_sb = [load(yv, [128, 1024], f"yv{i}", (i * 128, (i + 1) * 128)) for i in range(2)]

            # conv epilogue: psum has w@x; z = psum + bias (DVE), then
            # z*(1+tanh(z/2)) = 2*silu(z). Returns (z, u=z*tanh(z/2)); caller
            # emits the final add into the attention-dtype destination.
            def silu2(ps, bias_bc, tag, pcol=None):
                shape = list(ps.shape)
                z = actt.tile(shape, f32, tag="silu_z", name=f"z_{tag}")
                t = actt.tile(shape, f32, tag="silu_t", name=f"t_{tag}")
                u = actt.tile(shape, f32, tag="silu_u", name=f"u_{tag}")
                if pcol is not None:
                    nc.vector.tensor_scalar_add(z[:], ps, pcol)
                else:
                    nc.vector.tensor_add(z[:], ps, bias_bc)
                nc.scalar.activation(t[:], z[:], AF.Tanh, scale=0.5)
                nc.vector.tensor_mul(u[:], z[:], t[:])
                return z, u

            # ---- kv conv (k part): kT[pp, m0, m1], pp = hl*64+d ----
            kT = cp.tile([128, 8, 128], adt, tag="kT")
            for mt in range(4):  # m0 pairs
                ps = psc.tile([128, 512], f32, tag="cnv")
                for mi in range(2):
                    m0 = 2 * mt + mi
                    for c0 in range(2):
                        nc.tensor.matmul(
                            ps[:, mi * 128 : (mi + 1) * 128],
                            lhsT=yk_sb[c0][:, m0 * 128 : (m0 + 1) * 128],
                            rhs=wkv_sb[c0][:],
                            start=(c0 == 0), stop=(c0 == 1))
                z, u = silu2(ps[:, 0:256], bkv_bc[:], f"k{mt}")
                nc.vector.tensor_add(
                    kT[:, 2 * mt : 2 * mt + 2, :].rearrange("p a b -> p (a b)"),
                    z[:], u[:])

            # ---- kv conv (v part): vext[m1, m0, hl, 0:64]=2v, [...,64]=2 ----
            vext = cp.tile([128, 8, 2, 65], adt, tag="vext")
            nc.sync.dma_start(vext[:], twod.ap().partition_broadcast(128))
            for jv in range(2):
                ps = psc.tile([128, 512], f32, tag="cnv")
                for c0 in range(2):
                    nc.tensor.matmul(
                        ps[:], lhsT=wkv_sb[c0][:],
                        rhs=yv_sb[c0][:, jv * 512 : (jv + 1) * 512],
                        start=(c0 == 0), stop=(c0 == 1))
                z, u = silu2(ps[:], None, f"v{jv}", pcol=bkvc_sb[:])
                nc.vector.tensor_add(
                    vext[:, jv * 4 : (jv + 1) * 4, :, 0:64],
                    z[:].rearrange("p (a h d) -> p a h d", a=4, h=2),
                    u[:].rearrange("p (a h d) -> p a h d", a=4, h=2))

            # ---- q conv: qT[pp, n0, n1] ----
            qT = cp.tile([128, 16, 256], adt, tag="qT")
            for t2 in range(8):  # n0 pairs
                ps = psc.tile([128, 512], f32, tag="cnv")
                for nn in range(2):
                    n0 = 2 * t2 + nn
                    for c0 in range(2):
                        nc.tensor.matmul(
                            ps[:, nn * 256 : (nn + 1) * 256],
                            lhsT=xq_sb[c0][:, n0 * 128 : (n0 + 1) * 128],
                            rhs=wq_sb[c0][:],
                            start=(c0 == 0), stop=(c0 == 1))
                z, u = silu2(ps[:], bq_bc[:], f"q{t2}")
                nc.vector.tensor_add(
                    qT[:, 2 * t2 : 2 * t2 + 2, :].rearrange("p a b -> p (a b)"),
                    z[:], u[:])

            # ---- attention ----
            for hl in range(2):
                r0, r1 = hl * 64, (hl + 1) * 64
                sums_row = cp.tile([1, 4096], f32, tag=f"sumsrow{hl}",
                                   name=f"sumsrow{hl}")
                outun = [
                    cp.tile([128, 1024], f32, tag=f"outun{hl}_{i}",
                            name=f"outun{hl}_{i}")
                    for i in range(2)
                ]
                grid = [
                    cp.tile([128, 1024], f32, tag=f"grid{hl}_{i}",
                            name=f"grid{hl}_{i}")
                    for i in range(2)
                ]
                for t2 in range(8):  # n0 pairs (n0 = 2*t2 + nn)
                    att = []
                    for j in range(4):  # m0 = 2j + mi
                        scp = pss.tile([128, 1024], f32, tag="scp")
                        for mi in range(2):
                            m0 = 2 * j + mi
                            nc.tensor.matmul(
                                scp[:, mi * 512 : (mi + 1) * 512],
                                lhsT=kT[r0:r1, m0, :],
                                rhs=qT[r0:r1, 2 * t2 : 2 * t2 + 2, :],
                                start=True, stop=True)
                        a = attp.tile([128, 1024], adt, tag="att",
                                      name=f"att{hl}_{t2}_{j}")
                        # scoresT = 4*q.k ; want exp(q.k/8) -> scale 1/32
                        nc.scalar.activation(a[:], scp[:], AF.Exp, scale=0.03125)
                        att.append(a)
                    ops = pso.tile([65, 512], f32, tag="ops")
                    for m0 in range(8):
                        nc.tensor.matmul(
                            ops[:], lhsT=vext[:, m0, hl, :],
                            rhs=att[m0 // 2][:, (m0 % 2) * 512 : (m0 % 2 + 1) * 512],
                            start=(m0 == 0), stop=(m0 == 7))
                    nc.vector.tensor_copy(
                        sums_row[0:1, t2 * 512 : (t2 + 1) * 512], ops[64:65, :])
                    for nn in range(2):
                        n0 = 2 * t2 + nn
                        n3, q0 = n0 & 3, n0 >> 2
                        # o (x2) -> outun[c][band, q0-major cols] (contiguous)
                        dst = outun[n3 // 2][
                            (n3 % 2) * 64 : (n3 % 2) * 64 + 64,
                            q0 * 256 : (q0 + 1) * 256]
                        nc.vector.tensor_copy(
                            dst.bitcast(f32r),
                            ops[0:64, nn * 256 : (nn + 1) * 256])

                    if t2 % 2 == 0:
                        continue
                    # ---- quarter q0 = t2//2 complete: normalize + proj ----
                    q0 = t2 // 2
                    base = hl * 4096 + q0 * 1024
                    nc.sync.dma_start(
                        bass.AP(tensor=rraw, offset=base,
                                ap=[[1024, 1], [1, 1024]]),
                        sums_row[0:1, q0 * 1024 : (q0 + 1) * 1024])
                    srq = nrm.tile([16, 64], f32, tag="srq", name=f"srq{hl}_{q0}")
                    nc.sync.dma_start(
                        srq[:],
                        bass.AP(tensor=rraw, offset=base, ap=[[64, 16], [1, 64]]))
                    rcq = nrm.tile([16, 64], f32, tag="rcq", name=f"rcq{hl}_{q0}")
                    nc.vector.reciprocal(rcq[:], srq[:])
                    for n3g in range(4):
                        nc.sync.dma_start(
                            bass.AP(tensor=rscr,
                                    offset=hl * 4096 + n3g * 1024 + q0 * 256,
                                    ap=[[64, 4], [1, 64]]),
                            rcq[n3g * 4 : (n3g + 1) * 4, :])
                    for c0 in range(2):
                        for bnd in range(2):
                            n3g = c0 * 2 + bnd
                            src = bass.AP(
                                tensor=rscr,
                                offset=hl * 4096 + n3g * 1024 + q0 * 256,
                                ap=[[0, 64], [1, 256]])
                            nc.sync.dma_start(
                                grid[c0][bnd * 64 : (bnd + 1) * 64,
                                         q0 * 256 : (q0 + 1) * 256],
                                src)
                        sl = slice(q0 * 256, (q0 + 1) * 256)
                        nc.vector.tensor_mul(
                            outun[c0][:, sl].bitcast(f32r),
                            outun[c0][:, sl], grid[c0][:, sl])
                    # proj for fc in {2q0, 2q0+1}; rows hl*1024+half*512+q0+4r
                    ps = psc.tile([128, 512], f32, tag="cnv")
                    for half in range(2):
                        fc = 2 * q0 + half
                        for c0 in range(2):
                            nc.tensor.matmul(
                                ps[:, half * 256 : (half + 1) * 256],
                                lhsT=outun[c0][:].bitcast(f32r)[
                                    :, fc * 128 : (fc + 1) * 128],
                                rhs=wp_sb[c0][:],
                                start=(c0 == 0), stop=(c0 == 1))
                    osb = outp.tile([128, 512], f32, tag="osb",
                                    name=f"osb{hl}_{q0}")
                    nc.vector.tensor_add(osb[:], ps[:], bp_bc[:])
                    dstap = bass.AP(
                        tensor=out,
                        offset=(hl * 1024 + q0) * 256,
                        ap=[[4 * 256, 128], [512 * 256, 2], [1, 256]])
                    nc.sync.dma_start(
                        dstap,
                        osb[:].rearrange("p (h c) -> p h c", h=2))

    nc.compile()
    return nc


def _prep_inputs(x, y, q_w, q_gamma, q_beta, q_mean, q_var,
                 kv_w, kv_gamma, kv_beta, kv_mean, kv_var, proj_w, proj_b):
    f = np.float32
    x = np.ascontiguousarray(np.asarray(x, f))
    y = np.ascontiguousarray(np.asarray(y, f))

    gq = np.asarray(q_gamma, f) / np.sqrt(np.asarray(q_var, f) + BN_EPS)
    bq_full = np.asarray(q_beta, f) - np.asarray(q_mean, f) * gq
    wq_host = np.ascontiguousarray((np.asarray(q_w, f) * gq[:, None]).T)

    gkv = np.asarray(kv_gamma, f) / np.sqrt(np.asarray(kv_var, f) + BN_EPS)
    bkv_full = np.asarray(kv_beta, f) - np.asarray(kv_mean, f) * gkv
    wkv_host = np.ascontiguousarray((np.asarray(kv_w, f) * gkv[:, None]).T)

    wp_host = np.ascontiguousarray(np.asarray(proj_w, f).T)
    bp_host = np.asarray(proj_b, f)

    bq2 = np.tile(bq_full[None, :], (1, 2)).astype(f)
    bkv2 = np.tile(bkv_full[None, :], (1, 2)).astype(f)
    bp2 = np.tile(bp_host[None, :], (1, 2)).astype(f)

    in_maps = []
    for core in range(8):
        b, hp = core // 2, core % 2
        X4 = x[b].reshape(C, N_TOK)
        Y4 = y[b].reshape(C, N_TOK)
        xqa = np.ascontiguousarray(
            X4.reshape(C, 16, 256)[:, :, hp * 128 : (hp + 1) * 128]).reshape(C, 2048)
        Y8 = Y4.reshape(C, 8, 512)
        yka = np.ascontiguousarray(
            Y8[:, :, hp * 128 : (hp + 1) * 128]).reshape(C, 1024)
        yva = np.ascontiguousarray(
            Y8[:, :, 256 + hp * 128 : 256 + (hp + 1) * 128]).reshape(C, 1024)
        in_maps.append({
            "xq": xqa, "yk": yka, "yv": yva,
            "wq": wq_host, "bq": bq2,
            "wkv": wkv_host, "bkv": bkv2,
            "bkvc": bkv_full[:, None].astype(f),
            "wp": wp_host, "bp": bp2,
            "twod": np.full((1, 1040), 2.0,
                            ml_dtypes.bfloat16 if ATTN_BF16 else f),
        })
    return in_maps


def _get_nc():
    if "nc" not in _CACHE:
        _CACHE["nc"] = _build()
    return _CACHE["nc"]


def kernel(x, y, H=64, W=64, q_w=None, q_gamma=None, q_beta=None, q_mean=None,
           q_var=None, kv_w=None, kv_gamma=None, kv_beta=None, kv_mean=None,
           kv_var=None, proj_w=None, proj_b=None, _trace=False):
    from concourse.bass_utils import run_bass_kernel_spmd

    nc = _get_nc()
    in_maps = _prep_inputs(x, y, q_w, q_gamma, q_beta, q_mean, q_var,
                           kv_w, kv_gamma, kv_beta, kv_mean, kv_var,
                           proj_w, proj_b)
    kw = {}
    if _trace:
        kw = {"trace": True, "trace_cores": list(range(8))}
    res = run_bass_kernel_spmd(nc, in_maps, list(range(8)), **kw)
    outa = np.empty((B, N_TOK, C), np.float32)
    for core in range(8):
        b, hp = core // 2, core % 2
        outa[b, hp * 2048 : (hp + 1) * 2048, :] = res.results[core]["out"]
    if _trace:
        return outa, res
    return outa
